# revision 4
# baseline (speedup 1.0000x reference)
"""TRN2 Bass kernel for Llama-style prefill attention block.

Problem: B=2, S=2048, D=4096, H=32 q-heads, KVH=8 kv-heads, HD=128, causal
prefill with interleaved RoPE, GQA (n_rep=4), fp32 reference.

Sharding (8 NeuronCores): data-parallel over batch (2) x tensor-parallel over
heads (4): core c -> batch c//4, q-heads (c%4)*8..+8, kv-heads (c%4)*2..+2.
Each core computes a partial output [2048, 4096] (row-parallel wo) in bf16;
partials are summed on the host (4 cores per batch).

Per-core pipeline (all layouts chosen so NO on-chip transposes are needed):
  A) QKV projections from host-pretransposed x^T:
       K^T[hd, s] = (wk_h chunks)^T @ x^T    (per head, PSUM-accum over d)
       Q^T[hd, s] similarly; V[s, hd] = (x^T chunks)^T @ wv (fp16 on chip).
     RoPE applied during PSUM->SBUF eviction. Weights are host-permuted per
     head to [even cols | odd cols] so RoPE pairs become partition halves.
  B+C interleaved) Attention per head in scores^T layout, iterated g-major
     (q-block outer, head inner) with a software-pipeline skew so the
     in-order PE queue never waits on exp latency. Per (h, g) q-block the
     kernel walks k-tile pairs:
       scores^T[k, q] = K^T_chunk^T @ Q^T    (fp32r matmuls)
       es = exp(scores/sqrt(HD) - 2) on ACT as fp16 (the -2 bias guards the
       fp16 range and cancels exactly in the softmax ratio); causal 0/1
       [128,128] triangle mask multiplied post-exp on diagonal blocks only.
       Diagonal pairs are trimmed at 128 granularity (512+384 / 256+128
       columns) so almost no fully-masked work is issued.
       ctx^T[hd, q] += V_chunk^T @ es^T      (PSUM-accum over k-chunks)
       sacc[k, q]   += es                    (DVE fp16 adds, 2x mode)
     After a (h,g) block: gpsimd partition_all_reduce gives sums[128, q],
     DVE computes ctx^T * (1/sums) into ctx_sb as bf16. No PE cycles are
     spent on softmax denominators.
     Phase C (out[q,:] += ctx^T_h^T @ wo_h) is interleaved into the PE
     stream: C chunks for q-block g are emitted between the B supers of
     q-block g+1, filling the PE bubbles that exp throughput would
     otherwise cause, and keeping the PE p-state pinned high. wo streams
     through SBUF in [128, NH, 1024] quarters (double-buffered), re-read
     per q-block.

All DMA triggers are issued from the otherwise-idle SP (sync) queue (HWDGE).

Matmul dtypes: bf16 for the x/weight GEMMs (inputs pre-cast on host) and the
wo GEMM; float32r (full-rate fp32) for attention scores; fp16 for exp/PV.
"""
import sys
import math

sys.path.insert(0, "/opt/trn_rl_repo")

import numpy as np
import ml_dtypes

import concourse.bass as bass
import concourse.tile as tile
import concourse.mybir as mybir
from concourse import bacc
from concourse import bass_isa

F32 = mybir.dt.float32
F32R = mybir.dt.float32r
BF16 = mybir.dt.bfloat16
F16 = mybir.dt.float16
AF = mybir.ActivationFunctionType

B, S, D = 2, 2048, 4096
H, KVH, HD = 32, 8, 128
NH, NKV = 8, 2          # per-core q heads / kv heads
DC = D // 128           # 32 contraction chunks
NST = 4                 # phase-A s-tiles of 512
SB = S // NST           # 512
KT = S // 128           # 16 k chunks
INV_SQRT_HD = 1.0 / math.sqrt(HD)
EXP_BIAS = -2.0         # exp(x + bias): cancels in softmax, guards fp16

DT_X = BF16             # dtype of x^T and wq/wk/wv on chip


def build_kernel(phases=("A", "B", "C"), ablate=()):
    exp_fn = AF.Copy if "exp" in ablate else AF.Exp
    nc = bacc.Bacc(None, target_bir_lowering=False)

    xt = nc.dram_tensor("xt", [128, DC, S], DT_X, kind="ExternalInput")
    wq = nc.dram_tensor("wq", [NH, 128, DC, 128], DT_X, kind="ExternalInput")
    wk = nc.dram_tensor("wk", [NKV, 128, DC, 128], DT_X, kind="ExternalInput")
    wv = nc.dram_tensor("wv", [128, DC, NKV * 128], DT_X, kind="ExternalInput")
    wo = nc.dram_tensor("wo", [128, NH, D], BF16, kind="ExternalInput")
    cossin = nc.dram_tensor("cossin", [128, S], BF16, kind="ExternalInput")
    masktri = nc.dram_tensor("masktri", [128, 128], F16, kind="ExternalInput")
    out = nc.dram_tensor("out", [S, D], BF16, kind="ExternalOutput")

    with tile.TileContext(nc) as tc:
        # ---------------- persistent tiles ----------------
        with tc.tile_pool(name="persist", bufs=1) as persist:
            qt = persist.tile([128, NH * S], F32R)     # Q^T per head
            ebias = persist.tile([128, 1], F32)
            nc.vector.memset(ebias, EXP_BIAS)

            with tc.tile_pool(name="mid", bufs=1) as mid:
                kt_sb = mid.tile([128, NKV * S], F32R)
                v_sb = mid.tile([128, NKV, KT, 128], F16)

                # ================= Phase A: QKV projections =================
                with (
                    tc.tile_pool(name="xtp", bufs=2) as xtp,
                    tc.tile_pool(name="wqp", bufs=2) as wqp,
                    tc.tile_pool(name="wvp", bufs=1) as wvp,
                    tc.tile_pool(name="csp", bufs=1) as csp,
                    tc.tile_pool(name="rtmp", bufs=2) as rtmp,
                    tc.tile_pool(name="psA", bufs=3, space="PSUM") as psA,
                    tc.tile_pool(name="psV", bufs=2, space="PSUM") as psV,
                ):
                    wv_sb = wvp.tile([128, DC, NKV * 128], DT_X)
                    cs = csp.tile([128, S], BF16)   # rows 0:64 cos, 64:128 sin

                    def rope(dst_lo, dst_hi, ps, s0):
                        """dst = RoPE(ps) with [re|im] partition halves."""
                        c = cs[0:64, s0:s0 + SB]
                        sn = cs[64:128, s0:s0 + SB]
                        t = rtmp.tile([128, SB], F32, tag="t")
                        nc.vector.tensor_mul(t[0:64, :], ps[64:128, :], sn)
                        nc.vector.tensor_mul(t[64:128, :], ps[0:64, :], sn)
                        nc.vector.tensor_mul(dst_lo, ps[0:64, :], c)
                        nc.vector.tensor_sub(dst_lo, dst_lo, t[0:64, :])
                        nc.vector.tensor_mul(dst_hi, ps[64:128, :], c)
                        nc.vector.tensor_add(dst_hi, dst_hi, t[64:128, :])

                    for st in range(NST if "A" in phases else 0):
                        s0 = st * SB
                        xt_sb = xtp.tile([128, DC, SB], DT_X)
                        nxq = 4 if st == 0 else 2
                        xt_dmas = []
                        for xq in range(nxq):
                            d0 = xq * (DC // nxq)
                            xt_dmas.append((xt_sb[:, d0:d0 + DC // nxq, :],
                                            xt[:, d0:d0 + DC // nxq,
                                               s0:s0 + SB]))

                        wk_tiles = []
                        for kvh in range(NKV):
                            wk_tiles.append(wqp.tile([128, DC, 128], DT_X,
                                                     tag="w",
                                                     name=f"wk{kvh}"))
                        if st == 0:
                            # interleave wk0 / x^T chunks so the first K
                            # matmul chain starts as early as possible
                            nc.sync.dma_start(wk_tiles[0][:, 0:8, :],
                                              wk[0, :, 0:8, :])
                            nc.sync.dma_start(*xt_dmas[0])
                            nc.sync.dma_start(wk_tiles[0][:, 8:DC, :],
                                              wk[0, :, 8:DC, :])
                            nc.sync.dma_start(*xt_dmas[1])
                            nc.sync.dma_start(cs, cossin[:, :])
                            for xq in range(2, nxq):
                                nc.sync.dma_start(*xt_dmas[xq])
                            nc.sync.dma_start(wk_tiles[1], wk[1, :, :, :])
                        else:
                            nc.sync.dma_start(wk_tiles[0], wk[0, :, :, :])
                            for xq in range(nxq):
                                nc.sync.dma_start(*xt_dmas[xq])
                            nc.sync.dma_start(wk_tiles[1], wk[1, :, :, :])

                        # K^T projections + RoPE
                        for kvh in range(NKV):
                            ps = psA.tile([128, SB], F32)
                            for dc in range(DC):
                                nc.tensor.matmul(
                                    ps, wk_tiles[kvh][:, dc, :],
                                    xt_sb[:, dc, :],
                                    start=(dc == 0), stop=(dc == DC - 1))
                            col = kvh * S + s0
                            rope(kt_sb[0:64, col:col + SB],
                                 kt_sb[64:128, col:col + SB], ps, s0)

                        # Q^T projections + RoPE
                        for h in range(NH):
                            wq_sb = wqp.tile([128, DC, 128], DT_X, tag="w",
                                             name=f"wq{h}")
                            nc.sync.dma_start(wq_sb, wq[h, :, :, :])
                            if st == 0 and h == NH - 1:
                                nc.sync.dma_start(wv_sb, wv[:, :, :])
                            ps = psA.tile([128, SB], F32)
                            for dc in range(DC):
                                nc.tensor.matmul(
                                    ps, wq_sb[:, dc, :], xt_sb[:, dc, :],
                                    start=(dc == 0), stop=(dc == DC - 1))
                            col = h * S + s0
                            rope(qt[0:64, col:col + SB],
                                 qt[64:128, col:col + SB], ps, s0)

                        # V projections (natural layout, fp16 on chip)
                        for vc in range(SB // 128):
                            ps = psV.tile([128, NKV * 128], F32)
                            for dc in range(DC):
                                nc.tensor.matmul(
                                    ps, xt_sb[:, dc, vc * 128:(vc + 1) * 128],
                                    wv_sb[:, dc, :],
                                    start=(dc == 0), stop=(dc == DC - 1))
                            ktg = st * (SB // 128) + vc
                            nc.vector.tensor_copy(v_sb[:, :, ktg, :], ps)

                # ========= Phase B+C: attention + output projection =========
                NQW = 4
                QW = D // NQW                       # 1024 out-cols per quarter
                with (
                    tc.tile_pool(name="ctxp_sb", bufs=1) as ctxpool,
                    tc.tile_pool(name="mkb", bufs=1) as mkb,
                    tc.tile_pool(name="wop", bufs=2) as wop,
                    tc.tile_pool(name="esp", bufs=5) as esp,
                    tc.tile_pool(name="sap", bufs=2) as sap,
                    tc.tile_pool(name="smp", bufs=2) as smp,
                    tc.tile_pool(name="rcp", bufs=2) as rcp,
                    tc.tile_pool(name="stg", bufs=3) as stgp,
                    tc.tile_pool(name="scp", bufs=2, space="PSUM") as scp,
                    tc.tile_pool(name="ctxp", bufs=2, space="PSUM") as ctxp,
                    tc.tile_pool(name="psC", bufs=2, space="PSUM") as psC,
                ):
                    ctx_sb = ctxpool.tile([128, NH * S], BF16)
                    mk = mkb.tile([128, 128], F16)
                    nc.sync.dma_start(mk, masktri[:, :])

                    def load_woq(g, q):
                        t = wop.tile([128, NH, QW], BF16, tag="wo",
                                     name=f"woq{(g * NQW + q) % 2}")
                        for h in range(NH):
                            nc.sync.dma_start(
                                t[:, h, :],
                                wo[:, h, q * QW:(q + 1) * QW])
                        return t

                    # -------- B super-iterations, g-major ----------
                    iters = []
                    for g in range(4 if "B" in phases else 0):
                        for h in range(NH):
                            for kp in range(2 * g):
                                iters.append((h, g, "F", kp))
                            iters.append((h, g, "D0", None))
                            iters.append((h, g, "D1", None))
                    # index of last iter of each g (to release C work)
                    last_of_g = {}
                    for idx, (h, g, kind, kp) in enumerate(iters):
                        last_of_g[g] = idx

                    es_t = {}
                    cps = {}
                    sas = {}

                    def issue_front(i):
                        h, g, kind, kp = iters[i]
                        kvh = h // 4
                        q0 = h * S + g * SB
                        if (kind == "F" and kp == 0) or (kind == "D0"
                                                         and g == 0):
                            cps[(h, g)] = ctxp.tile([128, SB], F32,
                                                    tag="c", name="cp")
                            sas[(h, g)] = sap.tile([128, SB], F16,
                                                   tag="s", name="sa")
                        sc = scp.tile([128, 2 * SB], F32, tag="sc", name="sc")
                        es = esp.tile([128, 2 * SB], F16, tag="es", name="es")
                        if kind == "F":
                            kb = 2 * kp
                            for j in range(2):
                                k0 = kvh * S + (kb + j) * 128
                                nc.tensor.matmul(
                                    sc[:, j * SB:(j + 1) * SB],
                                    kt_sb[:, k0:k0 + 128],
                                    qt[:, q0:q0 + SB],
                                    start=True, stop=True)
                            nc.scalar.activation(out=es, in_=sc,
                                                 func=exp_fn,
                                                 scale=INV_SQRT_HD,
                                                 bias=ebias)
                        elif kind == "D0":
                            # k-tile 4g vs q[0:512); k-tile 4g+1 vs q[128:512)
                            k0 = kvh * S + 4 * g * 128
                            nc.tensor.matmul(sc[:, 0:SB],
                                             kt_sb[:, k0:k0 + 128],
                                             qt[:, q0:q0 + SB],
                                             start=True, stop=True)
                            nc.tensor.matmul(sc[:, SB:SB + 384],
                                             kt_sb[:, k0 + 128:k0 + 256],
                                             qt[:, q0 + 128:q0 + SB],
                                             start=True, stop=True)
                            nc.scalar.activation(out=es[:, 0:SB + 384],
                                                 in_=sc[:, 0:SB + 384],
                                                 func=exp_fn,
                                                 scale=INV_SQRT_HD,
                                                 bias=ebias)
                            nc.vector.tensor_mul(
                                es[:, 0:128], es[:, 0:128], mk)
                            nc.vector.tensor_mul(
                                es[:, SB:SB + 128], es[:, SB:SB + 128], mk)
                        else:  # D1: k-tile 4g+2 vs q[256:512); 4g+3 vs [384:)
                            k0 = kvh * S + (4 * g + 2) * 128
                            nc.tensor.matmul(sc[:, 0:256],
                                             kt_sb[:, k0:k0 + 128],
                                             qt[:, q0 + 256:q0 + SB],
                                             start=True, stop=True)
                            nc.tensor.matmul(sc[:, 256:384],
                                             kt_sb[:, k0 + 128:k0 + 256],
                                             qt[:, q0 + 384:q0 + SB],
                                             start=True, stop=True)
                            nc.scalar.activation(out=es[:, 0:384],
                                                 in_=sc[:, 0:384],
                                                 func=exp_fn,
                                                 scale=INV_SQRT_HD,
                                                 bias=ebias)
                            nc.vector.tensor_mul(
                                es[:, 0:128], es[:, 0:128], mk)
                            nc.vector.tensor_mul(
                                es[:, 256:384], es[:, 256:384], mk)
                        es_t[i] = es

                    def issue_back(i):
                        h, g, kind, kp = iters[i]
                        kvh = h // 4
                        es = es_t.pop(i)
                        cp = cps[(h, g)]
                        sa = sas[(h, g)]
                        first = (kind == "F" and kp == 0) or (kind == "D0"
                                                              and g == 0)
                        if kind == "F":
                            kb = 2 * kp
                            for j in range(2):
                                kt_i = kb + j
                                nc.tensor.matmul(cp,
                                                 v_sb[:, kvh, kt_i, :],
                                                 es[:, j * SB:(j + 1) * SB],
                                                 start=(kt_i == 0),
                                                 stop=False)
                            if first:
                                nc.vector.tensor_add(sa, es[:, 0:SB],
                                                     es[:, SB:2 * SB])
                            else:
                                nc.vector.tensor_add(sa, sa, es[:, 0:SB])
                                nc.vector.tensor_add(sa, sa, es[:, SB:2 * SB])
                        elif kind == "D0":
                            kt0 = 4 * g
                            nc.tensor.matmul(cp, v_sb[:, kvh, kt0, :],
                                             es[:, 0:SB],
                                             start=(kt0 == 0), stop=False)
                            nc.tensor.matmul(cp[:, 128:SB],
                                             v_sb[:, kvh, kt0 + 1, :],
                                             es[:, SB:SB + 384],
                                             start=False, stop=False)
                            if first:
                                nc.vector.tensor_copy(sa, es[:, 0:SB])
                            else:
                                nc.vector.tensor_add(sa, sa, es[:, 0:SB])
                            nc.vector.tensor_add(sa[:, 128:SB],
                                                 sa[:, 128:SB],
                                                 es[:, SB:SB + 384])
                        else:  # D1
                            kt0 = 4 * g + 2
                            nc.tensor.matmul(cp[:, 256:SB],
                                             v_sb[:, kvh, kt0, :],
                                             es[:, 0:256],
                                             start=False, stop=False)
                            nc.tensor.matmul(cp[:, 384:SB],
                                             v_sb[:, kvh, kt0 + 1, :],
                                             es[:, 256:384],
                                             start=False, stop=True)
                            nc.vector.tensor_add(sa[:, 256:SB],
                                                 sa[:, 256:SB], es[:, 0:256])
                            nc.vector.tensor_add(sa[:, 384:SB],
                                                 sa[:, 384:SB],
                                                 es[:, 256:384])
                            cps.pop((h, g))
                            sas.pop((h, g))
                            sm = smp.tile([128, SB], F32, tag="sm")
                            nc.gpsimd.partition_all_reduce(
                                sm, sa, channels=128,
                                reduce_op=bass_isa.ReduceOp.add)
                            rc = rcp.tile([128, SB], F32, tag="rc")
                            nc.vector.reciprocal(rc, sm)
                            q0 = h * S + g * SB
                            nc.vector.tensor_mul(
                                ctx_sb[:, q0:q0 + SB], cp, rc)

                    # -------- C chunk emission ----------
                    # one chunk = [128,512] out cols for one q-tile:
                    # accumulate 8 heads into one PSUM tile, evict, DMA out.
                    woq = {}

                    def emit_c_chunk(g, qi, quarter, half):
                        wot = woq[(g, quarter)]
                        ops = psC.tile([128, SB], F32, tag="o", name="op")
                        hs = slice(half * SB, (half + 1) * SB)
                        for h in range(NH):
                            q0 = h * S + qi * 128
                            nc.tensor.matmul(
                                ops, ctx_sb[:, q0:q0 + 128],
                                wot[:, h, hs],
                                start=(h == 0), stop=(h == NH - 1))
                        st_t = stgp.tile([128, SB], BF16)
                        nc.vector.tensor_copy(st_t, ops)
                        c0 = quarter * QW + half * SB
                        nc.sync.dma_start(
                            out[qi * 128:(qi + 1) * 128, c0:c0 + SB], st_t)

                    # C chunk schedule: chunks of q-block g interleave into
                    # the B supers of q-block g+1; C(3) trails at the end.
                    # Order: quarter-major so each wo quarter is loaded once
                    # per g.
                    def c_chunks_for(g):
                        for quarter in range(NQW):
                            for qi in range(4 * g, 4 * g + 4):
                                for half in range(2):
                                    yield (g, qi, quarter, half)

                    pending = []          # C chunks ready to emit
                    do_c = "C" in phases

                    if do_c and "B" not in phases:
                        # C-only ablation: emit everything directly
                        for g in range(4):
                            woq[(g, 0)] = load_woq(g, 0)
                            for (gg, qi, quarter, half) in c_chunks_for(g):
                                if half == 0 and qi == 4 * g and quarter + 1 < NQW:
                                    woq[(g, quarter + 1)] = load_woq(
                                        g, quarter + 1)
                                emit_c_chunk(gg, qi, quarter, half)

                    SKEW = 3
                    n_it = len(iters)
                    # per-iteration C emission rate while in g's supers
                    crate = {g: (32.0 / ((2 * g + 2) * 8)) for g in range(4)}
                    c_credit = 0.0
                    for i in range(n_it + SKEW):
                        if i < n_it:
                            issue_front(i)
                        if i >= SKEW:
                            j = i - SKEW
                            issue_back(j)
                            h, g, kind, kp = iters[j]
                            if do_c:
                                if j == last_of_g.get(g, -1):
                                    pending.extend(c_chunks_for(g))
                                    woq[(g, 0)] = load_woq(g, 0)
                                    c_credit = 0.0
                                if pending:
                                    pg = pending[0][0]
                                    c_credit += crate.get(pg + 1, 1.0)
                                    while c_credit >= 1.0 and pending:
                                        chunk = pending.pop(0)
                                        _, qi, quarter, half = chunk
                                        if (half == 0 and qi == 4 * chunk[0]
                                                and quarter + 1 < NQW):
                                            woq[(chunk[0], quarter + 1)] = \
                                                load_woq(chunk[0],
                                                         quarter + 1)
                                        emit_c_chunk(*chunk)
                                        c_credit -= 1.0
                    # drain remaining C chunks (C(3) + any leftovers)
                    for chunk in pending:
                        g, qi, quarter, half = chunk
                        if (g, quarter) not in woq:
                            woq[(g, quarter)] = load_woq(g, quarter)
                        elif half == 0 and qi == 4 * g and quarter + 1 < NQW:
                            if (g, quarter + 1) not in woq:
                                woq[(g, quarter + 1)] = load_woq(
                                    g, quarter + 1)
                        emit_c_chunk(g, qi, quarter, half)

    nc.finalize()
    return nc


# ---------------------------------------------------------------------------
# host-side prep + execution
# ---------------------------------------------------------------------------

_PERM = np.concatenate([np.arange(0, HD, 2), np.arange(1, HD, 2)])

_CACHE = {}


def _np_dt(dt):
    return ml_dtypes.bfloat16 if dt == BF16 else np.float32


def _prep_core_inputs(c, x, wq, wk, wv, wo, fc, fs, mask):
    b, g4 = c // 4, c % 4
    hq0, kv0 = g4 * 8, g4 * 2
    npx = _np_dt(DT_X)

    key = ("xt", b)
    if key not in _CACHE:
        xtv = np.ascontiguousarray(
            x[b].T.reshape(DC, 128, S).transpose(1, 0, 2)).astype(npx)
        _CACHE[key] = xtv
    xt = _CACHE[key]

    def wcols(w, head):  # [D, 128] -> [128, DC, 128]
        sl = w[:, head * 128:(head + 1) * 128][:, _PERM]
        return np.ascontiguousarray(
            sl.reshape(DC, 128, 128).transpose(1, 0, 2)).astype(npx)

    wq_c = np.stack([wcols(wq, hq0 + h) for h in range(NH)])
    wk_c = np.stack([wcols(wk, kv0 + kv) for kv in range(NKV)])
    wv_sl = wv[:, kv0 * 128:(kv0 + 2) * 128]
    wv_c = np.ascontiguousarray(
        wv_sl.reshape(DC, 128, NKV * 128).transpose(1, 0, 2)).astype(npx)
    wo_sl = wo[hq0 * 128:(hq0 + NH) * 128, :]
    wo_c = np.ascontiguousarray(
        wo_sl.reshape(NH, 128, D).transpose(1, 0, 2)).astype(ml_dtypes.bfloat16)

    key = "cossin"
    if key not in _CACHE:
        _CACHE[key] = np.ascontiguousarray(
            np.concatenate([fc.T, fs.T], axis=0)).astype(ml_dtypes.bfloat16)
    cossin = _CACHE[key]

    key = "masktri"
    if key not in _CACHE:
        # diagonal [128,128] block in scores^T layout: valid where k <= q,
        # i.e. partition p (k) <= column c (q)
        p = np.arange(128)
        _CACHE[key] = (p[:, None] <= p[None, :]).astype(np.float16)
    masktri = _CACHE[key]

    return dict(xt=xt, wq=wq_c, wk=wk_c, wv=wv_c, wo=wo_c,
                cossin=cossin, masktri=masktri)


def _reference_fallback(x, cache_k, cache_v, freqs_cos, freqs_sin, mask,
                        wq, wk, wv, wo, start_pos):
    """Pure-numpy fallback for inputs the fast path doesn't cover."""
    n_rep = H // KVH
    sp = int(start_pos)
    bsz, seqlen, _ = x.shape
    xq = (x @ wq).reshape(bsz, seqlen, H, HD)
    xk = (x @ wk).reshape(bsz, seqlen, KVH, HD)
    xv = (x @ wv).reshape(bsz, seqlen, KVH, HD)

    def rope_np(t):
        tr = t.reshape(*t.shape[:-1], HD // 2, 2)
        re, im = tr[..., 0], tr[..., 1]
        c = freqs_cos[None, :, None, :]
        s = freqs_sin[None, :, None, :]
        return np.stack([re * c - im * s, re * s + im * c],
                        axis=-1).reshape(t.shape).astype(np.float32)

    xq, xk = rope_np(xq), rope_np(xk)
    ck = np.array(cache_k)
    cv = np.array(cache_v)
    ck[:, sp:sp + seqlen] = xk
    cv[:, sp:sp + seqlen] = xv
    keys = np.repeat(ck[:, :sp + seqlen], n_rep, axis=2)
    vals = np.repeat(cv[:, :sp + seqlen], n_rep, axis=2)
    sc = np.einsum("bqhd,bkhd->bhqk", xq, keys) / np.sqrt(HD)
    sc = sc + mask[None, None, :sc.shape[2], :sc.shape[3]]
    sc = sc - sc.max(-1, keepdims=True)
    e = np.exp(sc)
    p = e / e.sum(-1, keepdims=True)
    ctx = np.einsum("bhqk,bkhd->bqhd", p, vals).reshape(bsz, seqlen, H * HD)
    return (ctx @ wo).astype(np.float32)


def _fast_path_ok(x, cache_k, cache_v, freqs_cos, freqs_sin, mask, wq, wk,
                  wv, wo, start_pos):
    if int(start_pos) != 0:
        return False
    if x.shape != (B, S, D) or mask.shape != (S, S):
        return False
    if np.any(cache_k) or np.any(cache_v):
        return False
    expect = np.triu(np.full((S, S), -1e9, dtype=np.float32), k=1)
    return np.array_equal(mask, expect)


def kernel(**inputs):
    x = np.asarray(inputs["x"], np.float32)
    cache_k = np.asarray(inputs["cache_k"], np.float32)
    cache_v = np.asarray(inputs["cache_v"], np.float32)
    fc = np.asarray(inputs["freqs_cos"], np.float32)
    fs = np.asarray(inputs["freqs_sin"], np.float32)
    mask = np.asarray(inputs["mask"], np.float32)
    wq = np.asarray(inputs["wq"], np.float32)
    wk = np.asarray(inputs["wk"], np.float32)
    wv = np.asarray(inputs["wv"], np.float32)
    wo = np.asarray(inputs["wo"], np.float32)
    start_pos = inputs["start_pos"]

    if not _fast_path_ok(x, cache_k, cache_v, fc, fs, mask, wq, wk, wv, wo,
                         start_pos):
        return _reference_fallback(x, cache_k, cache_v, fc, fs, mask,
                                   wq, wk, wv, wo, start_pos)

    # per-call input prep cache (cleared so repeat calls with new data
    # never reuse stale arrays; only the compiled runner persists)
    for k in [("xt", 0), ("xt", 1), "cossin", "masktri"]:
        _CACHE.pop(k, None)
    in_maps = [_prep_core_inputs(c, x, wq, wk, wv, wo, fc, fs, mask)
               for c in range(8)]
    results = _run(in_maps)

    out = np.zeros((B, S, D), np.float32)
    for c in range(8):
        out[c // 4] += results[c]["out"].astype(np.float32)
    return out


def _get_runner():
    if "runner" in _CACHE:
        return _CACHE["runner"]
    import jax
    from jax.sharding import Mesh, PartitionSpec
    from jax.experimental.shard_map import shard_map
    from concourse import bass2jax

    nc = build_kernel()
    bass2jax.install_neuronx_cc_hook()
    partition_name = (nc.partition_id_tensor.name
                      if nc.partition_id_tensor else None)
    in_names, out_names, out_avals, zero_outs = [], [], [], []
    for alloc in nc.m.functions[0].allocations:
        if not isinstance(alloc, mybir.MemoryLocationSet):
            continue
        name = alloc.memorylocations[0].name
        if alloc.kind == "ExternalInput":
            if name != partition_name:
                in_names.append(name)
        elif alloc.kind == "ExternalOutput":
            shape = tuple(alloc.tensor_shape)
            dtype = mybir.dt.np(alloc.dtype)
            out_avals.append(jax.core.ShapedArray(shape, dtype))
            out_names.append(name)
            zero_outs.append(np.zeros(shape, dtype))
    n_params = len(in_names)
    all_names = in_names + out_names
    if partition_name is not None:
        all_names.append(partition_name)

    def _body(*args):
        operands = list(args)
        if partition_name is not None:
            operands.append(bass2jax.partition_id_tensor())
        outs = bass2jax._bass_exec_p.bind(
            *operands,
            out_avals=tuple(out_avals),
            in_names=tuple(all_names),
            out_names=tuple(out_names),
            lowering_input_output_aliases=(),
            sim_require_finite=True,
            sim_require_nnan=True,
            nc=nc,
        )
        return tuple(outs)

    devices = jax.devices()[:8]
    mesh = Mesh(np.asarray(devices), ("core",))
    n_outs = len(out_names)
    in_specs = (PartitionSpec("core"),) * (n_params + n_outs)
    out_specs = (PartitionSpec("core"),) * n_outs
    fn = jax.jit(shard_map(_body, mesh=mesh, in_specs=in_specs,
                           out_specs=out_specs, check_rep=False),
                 keep_unused=True)
    runner = (fn, in_names, out_names, out_avals, zero_outs)
    _CACHE["runner"] = runner
    return runner


def _run(in_maps):
    import jax
    fn, in_names, out_names, out_avals, zero_outs = _get_runner()
    n_cores = len(in_maps)
    concat_in = [np.concatenate([np.asarray(in_maps[c][nm])
                                 for c in range(n_cores)], axis=0)
                 for nm in in_names]
    concat_zeros = [np.zeros((n_cores * z.shape[0], *z.shape[1:]), z.dtype)
                    for z in zero_outs]
    out_arrs = fn(*concat_in, *concat_zeros)
    return [
        {nm: np.asarray(out_arrs[i]).reshape(n_cores, *out_avals[i].shape)[c]
         for i, nm in enumerate(out_names)}
        for c in range(n_cores)
    ]


if __name__ == "__main__":
    # quick smoke: build + compile only
    nc = build_kernel()
    ni = len(nc.inst_map)
    print(f"built kernel: {ni} instructions")
